# revision 1
# baseline (speedup 1.0000x reference)
"""Trainium2 Bass kernel for CFContrastiveLoss.

Reference semantics (per sample of N=16 options, D=768 dims):
  - L2-normalize option embeddings
  - sim = pairwise cosine sims within the sample (16x16 gram)
  - max_neg[n] = max over negative-labeled columns of sim[n, :]
  - loss = mean over (positive rows of valid samples) of relu(max_neg + 0.3)

Device strategy (pure data parallel over batch, 8 cores):
  - 128 rows (= 8 samples x 16 options) per "group"; per core 16384 rows
    = 128 groups, DMA'd in super-groups of 8 (1.57 MB per transfer,
    alternating between the two HWDGE rings) for near-peak HBM bandwidth.
  - Host pre-normalizes embeddings, casts to fp16 and pre-transposes to
    the matmul layout.  The per-sample gram matrices are computed on the
    TensorEngine as block-diagonal 128x128 grams (fp32 PSUM accumulate).
    fp16 elements carry ~11 mantissa bits; the resulting per-sim error
    (~1e-5) averages out over the ~52k contributing rows, measured final
    loss error <= 2e-7 across seeds (fp32 matmuls would be 4x slower,
    bf16 hi+lo pairs need 2x the DMA bytes and 3x the matmuls).
  - The label/validity masking is folded into the same PSUM accumulation
    as one extra matmul of +-2^14 sentinel outer products (fp16-exact
    powers of two):
      row 0:  ones x negc           (negc[m] = -2^14 iff label[m] == 1)
      row 1:  ones x (-2^14 * ones) (mask everything ...)
      row 2+s: u_s x (+2^14 * u_s)  (... except within-sample blocks)
    Sentinels cancel exactly, so in-block negative columns get an
    exactly-zero mask contribution and unmasked sims are bit-exact;
    masked entries sit at <= -2^14 + 1 so relu(max + margin) = 0.
    Mask operands are zero-padded to K=128 so every matmul runs the same
    full-array tile config (mixed tile sizes cost PE reconfig stalls).
  - Per group the device then does a single VectorE row-max from PSUM.
    relu/margin/weighting/final mean are O(rows) and done on host.
"""

import os

import numpy as np

import concourse.bass as bass
import concourse.mybir as mybir
from concourse import bacc, tile
from concourse.bass_utils import run_bass_kernel_spmd

FP16 = mybir.dt.float16
F32 = mybir.dt.float32

B, N, D = 8192, 16, 768
N_CORES = 8
ROWS = B * N                      # 131072
ROWS_PER_CORE = ROWS // N_CORES   # 16384
GROUPS = ROWS_PER_CORE // 128     # 128 groups of 128 rows per core
KCH = D // 128                    # 6 contraction chunks
SG = 8                            # groups per super-group (one DMA batch)
N_SG = GROUPS // SG               # 16
MASK_K = 2 + 128 // N             # 10 live mask matmul rows
SENT = np.float32(2.0 ** 14)      # fp16-exact sentinel
MARGIN = np.float32(0.3)

_CACHE: dict = {}

LAST_RESULT = None  # BassKernelResults of the most recent device run


def _build_program() -> bass.Bass:
    nc = bacc.Bacc(None)
    et = nc.declare_dram_parameter("et", [N_SG, 128, SG * D], FP16, isOutput=False)
    mrhs = nc.declare_dram_parameter("mrhs", [N_SG, MASK_K, SG * 128], FP16, isOutput=False)
    mlhs = nc.declare_dram_parameter("mlhs", [128, 128], FP16, isOutput=False)
    out = nc.declare_dram_parameter("out", [N_SG, 128, SG], F32, isOutput=True)

    with tile.TileContext(nc) as tc:
        with (
            tc.tile_pool(name="emb", bufs=3) as emb_pool,
            tc.tile_pool(name="const", bufs=1) as const_pool,
            tc.tile_pool(name="wide", bufs=2) as wide_pool,
            tc.tile_pool(name="psum", bufs=8, space="PSUM") as psum_pool,
        ):
            mlhs_t = const_pool.tile([128, 128], FP16)
            nc.scalar.dma_start(mlhs_t[:], mlhs[:])
            # Two ping-pong mask-rhs tiles; pad rows (MASK_K..127) x zero
            # lhsT rows contribute nothing -- memset once so no NaN*0.
            mr_tiles = []
            for i in range(2):
                mr_t = const_pool.tile([128, SG * 128], FP16, name=f"mr{i}")
                nc.vector.memset(mr_t[:, :], 0.0)
                mr_tiles.append(mr_t)

            HALF = SG * D // 2
            for sg in range(N_SG):
                hi = emb_pool.tile([128, SG * D], FP16, tag="hi")
                mr_t = mr_tiles[sg % 2]
                if sg == 0:
                    # Tiny mask load first so the first mask matmul isn't
                    # queued behind the first big embedding transfer; load
                    # the first super-group in quarters so the leading
                    # groups' matmuls start as soon as their slice lands.
                    nc.sync.dma_start(mr_t[:MASK_K, :], mrhs[sg])
                    Q = HALF // 2
                    nc.sync.dma_start(hi[:, :Q], et[sg][:, :Q])
                    nc.scalar.dma_start(hi[:, HALF:HALF + Q], et[sg][:, HALF:HALF + Q])
                    nc.sync.dma_start(hi[:, Q:HALF], et[sg][:, Q:HALF])
                    nc.scalar.dma_start(hi[:, HALF + Q:], et[sg][:, HALF + Q:])
                else:
                    # Split each embedding load across both HWDGE rings so
                    # the two halves transfer in parallel.
                    nc.sync.dma_start(hi[:, :HALF], et[sg][:, :HALF])
                    nc.scalar.dma_start(hi[:, HALF:], et[sg][:, HALF:])
                    nc.sync.dma_start(mr_t[:MASK_K, :], mrhs[sg])
                wide = wide_pool.tile([128, SG], F32)
                for gi in range(SG):
                    ps = psum_pool.tile([128, 512], F32)  # one full PSUM bank
                    G = ps[:, 0:128]
                    # Mask sentinels first (start=True clears the bank).
                    nc.tensor.matmul(
                        G, mlhs_t[:], mr_t[:, gi * 128:(gi + 1) * 128],
                        start=True, stop=False,
                    )
                    for k in range(KCH):
                        c0 = (gi * KCH + k) * 128
                        hk = hi[:, c0:c0 + 128]
                        nc.tensor.matmul(G, hk, hk, start=False, stop=(k == KCH - 1))
                    nc.vector.reduce_max(wide[:, gi:gi + 1], G, axis=mybir.AxisListType.X)
                nc.scalar.dma_start(out[sg], wide[:])
    nc.finalize()
    return nc


def _prep_core_inputs(Xn16: np.ndarray, lab: np.ndarray):
    """Per-core input maps from fp16-normalized embeddings + flat labels."""
    negc = np.where(lab == 1, -SENT, np.float32(0.0)).astype(np.float16)

    m_idx = np.arange(128)
    # mask lhsT: [128, 128], rows = [ones; ones; u_0..u_7; zeros...]
    mlhs = np.zeros((128, 128), dtype=np.float16)
    mlhs[0, :] = 1.0
    mlhs[1, :] = 1.0
    for s in range(128 // N):
        mlhs[2 + s, :] = (m_idx // N == s).astype(np.float16)

    # static part of mask rhs rows 1..9 (per 128-column group)
    mrhs_static = np.zeros((MASK_K, 128), dtype=np.float16)
    mrhs_static[1, :] = -SENT
    for s in range(128 // N):
        mrhs_static[2 + s, :] = np.where(m_idx // N == s, SENT, 0.0).astype(np.float16)

    def to_sg_layout(a_core: np.ndarray) -> np.ndarray:
        # [rows=16384, 768] -> [sg, gi, n, k, p] -> [sg, p, gi, k, n]
        return np.ascontiguousarray(
            a_core.reshape(N_SG, SG, 128, KCH, 128).transpose(0, 4, 1, 3, 2)
        ).reshape(N_SG, 128, SG * D)

    in_maps = []
    for c in range(N_CORES):
        r0 = c * ROWS_PER_CORE
        r1 = r0 + ROWS_PER_CORE
        # mask rhs per super-group: [N_SG, 10, SG*128]
        mr = np.empty((MASK_K, ROWS_PER_CORE), dtype=np.float16)
        mr[0, :] = negc[r0:r1]
        mr[1:, :] = np.tile(mrhs_static[1:, :], (1, GROUPS))
        mr = mr.reshape(MASK_K, N_SG, SG * 128).transpose(1, 0, 2)
        in_maps.append({
            "et": to_sg_layout(Xn16[r0:r1]),
            "mrhs": np.ascontiguousarray(mr),
            "mlhs": mlhs,
        })
    return in_maps


def kernel(embeddings: np.ndarray, labels: np.ndarray) -> np.ndarray:
    global LAST_RESULT
    assert embeddings.shape == (B, N, D)
    assert labels.shape == (B, N)

    X = np.asarray(embeddings, dtype=np.float32).reshape(ROWS, D)
    lab = np.asarray(labels).reshape(ROWS)

    ss = np.square(X).sum(axis=1, dtype=np.float32)
    norms = np.sqrt(ss)
    Xn16 = (X / np.maximum(norms, np.float32(1e-12))[:, None]).astype(np.float16)

    in_maps = _prep_core_inputs(Xn16, lab)

    if "nc" not in _CACHE:
        _CACHE["nc"] = _build_program()
    nc = _CACHE["nc"]

    trace = os.environ.get("BASS_KERNEL_TRACE", "0") == "1"
    res = run_bass_kernel_spmd(nc, in_maps, list(range(N_CORES)), trace=trace)
    LAST_RESULT = res

    # out[sg, p, gi]: group g = sg*SG+gi, row-within-group p
    maxneg = np.concatenate(
        [np.asarray(r["out"]).transpose(0, 2, 1).reshape(ROWS_PER_CORE)
         for r in res.results]
    )

    triplet = np.maximum(maxneg + MARGIN, np.float32(0.0))
    has_neg = (np.asarray(labels) == 0).any(axis=1)
    w = (lab == 1) & np.repeat(has_neg, N)
    loss_sum = np.float32((triplet * w).sum(dtype=np.float64))
    count = int(w.sum())
    loss = np.float32(loss_sum / np.float32(max(count, 1)))
    return np.asarray(loss, dtype=np.float32)



# revision 2
# speedup vs baseline: 1.3156x; 1.3156x over previous
"""Trainium2 Bass kernel for CFContrastiveLoss.

Reference semantics (per sample of N=16 options, D=768 dims):
  - L2-normalize option embeddings
  - sim = pairwise cosine sims within the sample (16x16 gram)
  - max_neg[n] = max over negative-labeled columns of sim[n, :]
  - loss = mean over (positive rows of valid samples) of relu(max_neg + 0.3)

Device strategy (pure data parallel over batch, 8 cores):
  - 128 rows (= 8 samples x 16 options) per "group"; per core 16384 rows
    = 128 groups, DMA'd in super-groups of 8 (0.79 MB per transfer,
    alternating between the two HWDGE rings) for near-peak HBM bandwidth.
  - Host pre-normalizes embeddings, scales by 16 (power of two; keeps
    elements inside e4m3's normal range) and casts to fp8 e4m3 in the
    matmul layout.  This is a memory-bound problem, so fp8 halves the
    HBM traffic vs fp16.  Per-sample gram matrices are computed on the
    TensorEngine as block-diagonal 128x128 grams (fp32 PSUM accumulate)
    using DoubleRow fp8 matmuls: each matmul consumes TWO 128-row
    k-subtiles at the double-pumped fp8 rate, so the 768-dim contraction
    is 3 matmuls instead of 6.  Sims come out scaled by 256; the host
    divides it back out.  e4m3 carries 3 mantissa bits; the per-sim
    error (~2e-3 absolute) averages out over the ~52k contributing rows
    and the max() bias stays small because top-sim gaps are larger than
    the noise.  Measured final loss error ~1.4e-4 (threshold 2e-2).
  - The label/validity masking is folded into the same PSUM accumulation
    as one extra DoubleRow matmul of +-2^14 sentinel outer products
    (every factor is +-128 or 0, all fp8-exact):
      row 0:  128*ones x negc       (negc[m] = -128 iff label[m] == 1)
      row 1:  128*ones x (-128*ones)  (mask everything ...)
      row 2+s: 128*u_s x (128*u_s)    (... except within-sample blocks)
    Sentinels are exact powers of two so they cancel exactly in fp32:
    in-block negative columns get an exactly-zero mask contribution and
    unmasked sims are bit-exact; masked entries sit at <= -2^14 + 256 so
    relu(max/256 + margin) = 0.  Mask operands are zero-padded to the
    full [128, 2, 128] DoubleRow shape so every matmul runs the same
    tile config (mixed tile configs cost PE reconfig stalls).
  - Per group the device then does a single VectorE row-max from PSUM.
    relu/margin/weighting/final mean are O(rows) and done on host.
"""

import os

import ml_dtypes
import numpy as np

import concourse.bass as bass
import concourse.mybir as mybir
from concourse import bacc, tile
from concourse.bass_utils import run_bass_kernel_spmd

FP8 = mybir.dt.float8e4
NP_FP8 = ml_dtypes.float8_e4m3
F32 = mybir.dt.float32
DOUBLE_ROW = mybir.MatmulPerfMode.DoubleRow

B, N, D = 8192, 16, 768
N_CORES = 8
ROWS = B * N                      # 131072
ROWS_PER_CORE = ROWS // N_CORES   # 16384
GROUPS = ROWS_PER_CORE // 128     # 128 groups of 128 rows per core
KCH = D // 128                    # 6 contraction chunks
SG = 8                            # groups per super-group (one DMA batch)
N_SG = GROUPS // SG               # 16
MASK_K = 2 + 128 // N             # 10 live mask matmul rows
SENT = np.float32(128.0)          # fp8-exact sentinel factor (128*128 = 2^14)
ESCALE = np.float32(16.0)         # fp8 embedding scale (power of two)
SIM_SCALE = ESCALE * ESCALE       # gram outputs are scaled by this
MARGIN = np.float32(0.3)

_CACHE: dict = {}

LAST_RESULT = None  # BassKernelResults of the most recent device run


def _build_program() -> bass.Bass:
    nc = bacc.Bacc(None)
    et = nc.declare_dram_parameter("et", [N_SG, 128, SG * KCH, 128], FP8, isOutput=False)
    mrhs = nc.declare_dram_parameter("mrhs", [N_SG, MASK_K, SG * 128], FP8, isOutput=False)
    mlhs = nc.declare_dram_parameter("mlhs", [128, 128], FP8, isOutput=False)
    out = nc.declare_dram_parameter("out", [N_SG, 128, SG], F32, isOutput=True)

    with tile.TileContext(nc) as tc:
        with (
            tc.tile_pool(name="emb", bufs=3) as emb_pool,
            tc.tile_pool(name="const", bufs=1) as const_pool,
            tc.tile_pool(name="wide", bufs=2) as wide_pool,
            tc.tile_pool(name="psum", bufs=8, space="PSUM") as psum_pool,
        ):
            # Mask lhsT as a [128, 2, 128] DoubleRow weight tensor;
            # k-subtile 1 is all zeros (contributes nothing).
            mlhs_t = const_pool.tile([128, 2, 128], FP8)
            nc.vector.memset(mlhs_t[:, :, :], 0.0)
            nc.scalar.dma_start(mlhs_t[:, 0, :], mlhs[:])
            # Two ping-pong mask-rhs tiles; rows (MASK_K..127) of subtile
            # 0 and all of subtile 1 stay zero -- memset once so the
            # padded lanes contribute nothing (and no NaN*0).
            mr_tiles = []
            for i in range(2):
                mr_t = const_pool.tile([128, 2, SG * 128], FP8, name=f"mr{i}")
                nc.vector.memset(mr_t[:, :, :], 0.0)
                mr_tiles.append(mr_t)

            HALFC = SG * KCH // 2
            for sg in range(N_SG):
                hi = emb_pool.tile([128, SG * KCH, 128], FP8, tag="hi")
                mr_t = mr_tiles[sg % 2]
                if sg == 0:
                    # Tiny mask load first so the first mask matmul isn't
                    # queued behind the first big embedding transfer; load
                    # the first super-group in quarters so the leading
                    # groups' matmuls start as soon as their slice lands.
                    nc.sync.dma_start(mr_t[:MASK_K, 0, :], mrhs[sg])
                    Q = HALFC // 2
                    nc.sync.dma_start(hi[:, :Q, :], et[sg][:, :Q, :])
                    nc.scalar.dma_start(hi[:, HALFC:HALFC + Q, :], et[sg][:, HALFC:HALFC + Q, :])
                    nc.sync.dma_start(hi[:, Q:HALFC, :], et[sg][:, Q:HALFC, :])
                    nc.scalar.dma_start(hi[:, HALFC + Q:, :], et[sg][:, HALFC + Q:, :])
                else:
                    # Split each embedding load across both HWDGE rings so
                    # the two halves transfer in parallel.
                    nc.sync.dma_start(hi[:, :HALFC, :], et[sg][:, :HALFC, :])
                    nc.scalar.dma_start(hi[:, HALFC:, :], et[sg][:, HALFC:, :])
                    nc.sync.dma_start(mr_t[:MASK_K, 0, :], mrhs[sg])
                wide = wide_pool.tile([128, SG], F32)
                for gi in range(SG):
                    ps = psum_pool.tile([128, 512], F32)  # one full PSUM bank
                    G = ps[:, 0:128]
                    # Mask sentinels first (start=True clears the bank).
                    nc.tensor.matmul(
                        G, mlhs_t[:, :, :], mr_t[:, :, gi * 128:(gi + 1) * 128],
                        start=True, stop=False, perf_mode=DOUBLE_ROW,
                    )
                    for k in range(KCH // 2):
                        c0 = gi * KCH + 2 * k
                        hk = hi[:, c0:c0 + 2, :]
                        nc.tensor.matmul(
                            G, hk, hk,
                            start=False, stop=(k == KCH // 2 - 1),
                            perf_mode=DOUBLE_ROW,
                        )
                    nc.vector.reduce_max(wide[:, gi:gi + 1], G, axis=mybir.AxisListType.X)
                nc.scalar.dma_start(out[sg], wide[:])
    nc.finalize()
    return nc


def _prep_core_inputs(Xn8: np.ndarray, lab: np.ndarray):
    """Per-core input maps from fp8-normalized embeddings + flat labels."""
    negc = np.where(lab == 1, -SENT, np.float32(0.0)).astype(NP_FP8)

    m_idx = np.arange(128)
    # mask lhsT: [128, 128], rows = SENT * [ones; ones; u_0..u_7; zeros...]
    mlhs = np.zeros((128, 128), dtype=NP_FP8)
    mlhs[0, :] = SENT
    mlhs[1, :] = SENT
    for s in range(128 // N):
        mlhs[2 + s, :] = (SENT * (m_idx // N == s)).astype(NP_FP8)

    # static part of mask rhs rows 1..9 (per 128-column group)
    mrhs_static = np.zeros((MASK_K, 128), dtype=NP_FP8)
    mrhs_static[1, :] = -SENT
    for s in range(128 // N):
        mrhs_static[2 + s, :] = np.where(m_idx // N == s, SENT, np.float32(0.0)).astype(NP_FP8)

    def to_sg_layout(a_core: np.ndarray) -> np.ndarray:
        # [rows=16384, 768] -> [sg, gi, n, k, p] -> [sg, p, gi, k, n]
        return np.ascontiguousarray(
            a_core.reshape(N_SG, SG, 128, KCH, 128).transpose(0, 4, 1, 3, 2)
        ).reshape(N_SG, 128, SG * KCH, 128)

    in_maps = []
    for c in range(N_CORES):
        r0 = c * ROWS_PER_CORE
        r1 = r0 + ROWS_PER_CORE
        # mask rhs per super-group: [N_SG, 10, SG*128]
        mr = np.empty((MASK_K, ROWS_PER_CORE), dtype=NP_FP8)
        mr[0, :] = negc[r0:r1]
        mr[1:, :] = np.tile(mrhs_static[1:, :], (1, GROUPS))
        mr = mr.reshape(MASK_K, N_SG, SG * 128).transpose(1, 0, 2)
        in_maps.append({
            "et": to_sg_layout(Xn8[r0:r1]),
            "mrhs": np.ascontiguousarray(mr),
            "mlhs": mlhs,
        })
    return in_maps


def kernel(embeddings: np.ndarray, labels: np.ndarray) -> np.ndarray:
    global LAST_RESULT
    assert embeddings.shape == (B, N, D)
    assert labels.shape == (B, N)

    X = np.asarray(embeddings, dtype=np.float32).reshape(ROWS, D)
    lab = np.asarray(labels).reshape(ROWS)

    ss = np.square(X).sum(axis=1, dtype=np.float32)
    norms = np.sqrt(ss)
    Xn8 = (X * (ESCALE / np.maximum(norms, np.float32(1e-12)))[:, None]).astype(NP_FP8)

    in_maps = _prep_core_inputs(Xn8, lab)

    if "nc" not in _CACHE:
        _CACHE["nc"] = _build_program()
    nc = _CACHE["nc"]

    trace = os.environ.get("BASS_KERNEL_TRACE", "0") == "1"
    res = run_bass_kernel_spmd(nc, in_maps, list(range(N_CORES)), trace=trace)
    LAST_RESULT = res

    # out[sg, p, gi]: group g = sg*SG+gi, row-within-group p
    maxneg = np.concatenate(
        [np.asarray(r["out"]).transpose(0, 2, 1).reshape(ROWS_PER_CORE)
         for r in res.results]
    )

    triplet = np.maximum(maxneg / SIM_SCALE + MARGIN, np.float32(0.0))
    has_neg = (np.asarray(labels) == 0).any(axis=1)
    w = (lab == 1) & np.repeat(has_neg, N)
    loss_sum = np.float32((triplet * w).sum(dtype=np.float64))
    count = int(w.sum())
    loss = np.float32(loss_sum / np.float32(max(count, 1)))
    return np.asarray(loss, dtype=np.float32)


# revision 4
# speedup vs baseline: 1.5227x; 1.1575x over previous
"""Trainium2 Bass kernel for CFContrastiveLoss.

Reference semantics (per sample of N=16 options, D=768 dims):
  - L2-normalize option embeddings
  - sim = pairwise cosine sims within the sample (16x16 gram)
  - max_neg[n] = max over negative-labeled columns of sim[n, :]
  - loss = mean over (positive rows of valid samples) of relu(max_neg + 0.3)

Device strategy (pure data parallel over batch, 8 cores):
  - 128 rows (= 8 samples x 16 options) per "group"; per core 16384 rows
    = 128 groups, DMA'd in super-groups of 16 (1.57 MB per sg, split
    across the two HWDGE rings as 6144 B per-partition runs) for
    near-peak HBM bandwidth at few descriptors.
  - Host pre-normalizes embeddings, scales by 16 (power of two; keeps
    elements inside e4m3's normal range) and casts to fp8 e4m3 in the
    matmul layout.  This is a memory-bound problem, so fp8 halves the
    HBM traffic vs fp16.  Per-sample gram matrices are computed on the
    TensorEngine as block-diagonal 128x128 grams (fp32 PSUM accumulate)
    using DoubleRow fp8 matmuls: each matmul consumes TWO 128-row
    k-subtiles at the double-pumped fp8 rate, so the 768-dim contraction
    is 3 matmuls instead of 6.  Sims come out scaled by 256; the host
    divides it back out.  e4m3 carries 3 mantissa bits; the per-sim
    error (~2e-3 absolute) averages out over the ~52k contributing rows
    and the max() bias stays small because top-sim gaps are larger than
    the noise.  Measured final loss error ~1.4e-4 (threshold 2e-2).
  - The label/validity masking is folded into the same PSUM accumulation
    as sentinel outer-product matmuls of +-2^14 (every factor is +-128
    or 0, all fp8-exact):
      row 0:  128*ones x negc       (negc[m] = -128 iff label[m] == 1)
      row 1:  128*ones x (-128*ones)  (mask everything ...)
      row 2+s: 128*u_s x (128*u_s)    (... except within-sample blocks)
    Sentinels are exact powers of two so they cancel exactly in fp32:
    in-block negative columns get an exactly-zero mask contribution and
    unmasked sims are bit-exact; masked entries sit at <= -2^14 + 256 so
    relu(max/256 + margin) = 0.  One 512-column mask matmul seeds a
    whole PSUM bank (start=True) for 4 groups at once; the per-group
    gram matmuls then accumulate into their 128-column slice of the
    bank.  Mask operands are zero-padded to the full [128, 2, *]
    DoubleRow shape so every matmul runs the same tile config (mixed
    tile configs cost PE reconfig stalls).
  - Per bank the device then does a single VectorE row-max from PSUM
    over a [128, 4, 128] view (reduces the innermost axis -> [128, 4]),
    amortizing DVE instruction overhead over 4 groups.  All row maxes
    accumulate in one [128, 128] SBUF tile, stored with a single DMA at
    the end.  relu/margin/weighting/final mean are O(rows) on host.
"""

import os

import ml_dtypes
import numpy as np

import concourse.bass as bass
import concourse.mybir as mybir
from concourse import bacc, tile
from concourse.bass_utils import run_bass_kernel_spmd

FP8 = mybir.dt.float8e4
NP_FP8 = ml_dtypes.float8_e4m3
F32 = mybir.dt.float32
DOUBLE_ROW = mybir.MatmulPerfMode.DoubleRow

B, N, D = 8192, 16, 768
N_CORES = 8
ROWS = B * N                      # 131072
ROWS_PER_CORE = ROWS // N_CORES   # 16384
GROUPS = ROWS_PER_CORE // 128     # 128 groups of 128 rows per core
KCH = D // 128                    # 6 contraction chunks
SG = 16                           # groups per super-group (one DMA batch)
N_SG = GROUPS // SG               # 8
GPB = 4                           # groups per PSUM bank (512 f32 / 128)
BPSG = SG // GPB                  # PSUM banks per super-group
MASK_K = 2 + 128 // N             # 10 live mask matmul rows
SENT = np.float32(128.0)          # fp8-exact sentinel factor (128*128 = 2^14)
ESCALE = np.float32(16.0)         # fp8 embedding scale (power of two)
SIM_SCALE = ESCALE * ESCALE       # gram outputs are scaled by this
MARGIN = np.float32(0.3)

_CACHE: dict = {}

LAST_RESULT = None  # BassKernelResults of the most recent device run


def _build_program() -> bass.Bass:
    nc = bacc.Bacc(None)
    et = nc.declare_dram_parameter("et", [N_SG, 128, SG * KCH, 128], FP8, isOutput=False)
    mrhs = nc.declare_dram_parameter("mrhs", [N_SG, MASK_K, SG * 128], FP8, isOutput=False)
    mlhs = nc.declare_dram_parameter("mlhs", [128, 128], FP8, isOutput=False)
    out = nc.declare_dram_parameter("out", [128, GROUPS], F32, isOutput=True)

    with tile.TileContext(nc) as tc:
        with (
            tc.tile_pool(name="emb", bufs=3) as emb_pool,
            tc.tile_pool(name="const", bufs=1) as const_pool,
            tc.tile_pool(name="psum", bufs=8, space="PSUM") as psum_pool,
        ):
            # Mask lhsT as a [128, 2, 128] DoubleRow weight tensor;
            # k-subtile 1 is all zeros (contributes nothing).
            mlhs_t = const_pool.tile([128, 2, 128], FP8)
            nc.vector.memset(mlhs_t[:, :, :], 0.0)
            nc.scalar.dma_start(mlhs_t[:, 0, :], mlhs[:])
            # Two ping-pong mask-rhs tiles; rows (MASK_K..127) of subtile
            # 0 and all of subtile 1 stay zero -- memset once so the
            # padded lanes contribute nothing (and no NaN*0).
            mr_tiles = []
            for i in range(2):
                mr_t = const_pool.tile([128, 2, SG * 128], FP8, name=f"mr{i}")
                nc.vector.memset(mr_t[:, :, :], 0.0)
                mr_tiles.append(mr_t)
            # All 128 row-maxes accumulate here; one DMA at the end.
            wide = const_pool.tile([128, GROUPS], F32)

            HALFC = SG * KCH // 2
            for sg in range(N_SG):
                hi = emb_pool.tile([128, SG * KCH, 128], FP8, tag="hi")
                mr_t = mr_tiles[sg % 2]
                if sg == 0:
                    # Tiny mask load first so the first mask matmul isn't
                    # queued behind the first big embedding transfer; load
                    # the first super-group in eighths so the leading
                    # groups' matmuls start as soon as their slice lands.
                    nc.sync.dma_start(mr_t[:MASK_K, 0, :], mrhs[sg])
                    Q = HALFC // 4
                    for q in range(4):
                        nc.sync.dma_start(
                            hi[:, q * Q:(q + 1) * Q, :], et[sg][:, q * Q:(q + 1) * Q, :])
                        nc.scalar.dma_start(
                            hi[:, HALFC + q * Q:HALFC + (q + 1) * Q, :],
                            et[sg][:, HALFC + q * Q:HALFC + (q + 1) * Q, :])
                else:
                    # Split each embedding load across both HWDGE rings so
                    # the two halves transfer in parallel.
                    nc.sync.dma_start(hi[:, :HALFC, :], et[sg][:, :HALFC, :])
                    nc.scalar.dma_start(hi[:, HALFC:, :], et[sg][:, HALFC:, :])
                    nc.sync.dma_start(mr_t[:MASK_K, 0, :], mrhs[sg])
                for b in range(BPSG):
                    ps = psum_pool.tile([128, GPB, 128], F32)  # one full PSUM bank
                    # Mask sentinels for 4 groups at once (start=True
                    # clears the whole bank).
                    c0 = b * GPB * 128
                    nc.tensor.matmul(
                        ps[:, :, :], mlhs_t[:, :, :], mr_t[:, :, c0:c0 + GPB * 128],
                        start=True, stop=False, perf_mode=DOUBLE_ROW,
                    )
                    for g in range(GPB):
                        gi = b * GPB + g
                        G = ps[:, g, :]
                        for k in range(KCH // 2):
                            kc = gi * KCH + 2 * k
                            hk = hi[:, kc:kc + 2, :]
                            nc.tensor.matmul(
                                G, hk, hk,
                                start=False, stop=(k == KCH // 2 - 1),
                                perf_mode=DOUBLE_ROW,
                            )
                    # One DVE reduce for the whole bank: [128, 4, 128]
                    # reduced over the innermost axis -> [128, 4].
                    nc.vector.reduce_max(
                        wide[:, sg * SG + b * GPB: sg * SG + (b + 1) * GPB],
                        ps[:, :, :], axis=mybir.AxisListType.X)
            nc.sync.dma_start(out[:, :], wide[:, :])
    nc.finalize()
    return nc


def _prep_core_inputs(Xn8: np.ndarray, lab: np.ndarray):
    """Per-core input maps from fp8-normalized embeddings + flat labels."""
    negc = np.where(lab == 1, -SENT, np.float32(0.0)).astype(NP_FP8)

    m_idx = np.arange(128)
    # mask lhsT: [128, 128], rows = SENT * [ones; ones; u_0..u_7; zeros...]
    mlhs = np.zeros((128, 128), dtype=NP_FP8)
    mlhs[0, :] = SENT
    mlhs[1, :] = SENT
    for s in range(128 // N):
        mlhs[2 + s, :] = (SENT * (m_idx // N == s)).astype(NP_FP8)

    # static part of mask rhs rows 1..9 (per 128-column group)
    mrhs_static = np.zeros((MASK_K, 128), dtype=NP_FP8)
    mrhs_static[1, :] = -SENT
    for s in range(128 // N):
        mrhs_static[2 + s, :] = np.where(m_idx // N == s, SENT, np.float32(0.0)).astype(NP_FP8)

    def to_sg_layout(a_core: np.ndarray) -> np.ndarray:
        # [rows=16384, 768] -> [sg, gi, n, k, p] -> [sg, p, gi, k, n]
        return np.ascontiguousarray(
            a_core.reshape(N_SG, SG, 128, KCH, 128).transpose(0, 4, 1, 3, 2)
        ).reshape(N_SG, 128, SG * KCH, 128)

    in_maps = []
    for c in range(N_CORES):
        r0 = c * ROWS_PER_CORE
        r1 = r0 + ROWS_PER_CORE
        # mask rhs per super-group: [N_SG, 10, SG*128]
        mr = np.empty((MASK_K, ROWS_PER_CORE), dtype=NP_FP8)
        mr[0, :] = negc[r0:r1]
        mr[1:, :] = np.tile(mrhs_static[1:, :], (1, GROUPS))
        mr = mr.reshape(MASK_K, N_SG, SG * 128).transpose(1, 0, 2)
        in_maps.append({
            "et": to_sg_layout(Xn8[r0:r1]),
            "mrhs": np.ascontiguousarray(mr),
            "mlhs": mlhs,
        })
    return in_maps


def kernel(embeddings: np.ndarray, labels: np.ndarray) -> np.ndarray:
    global LAST_RESULT
    assert embeddings.shape == (B, N, D)
    assert labels.shape == (B, N)

    X = np.asarray(embeddings, dtype=np.float32).reshape(ROWS, D)
    lab = np.asarray(labels).reshape(ROWS)

    ss = np.square(X).sum(axis=1, dtype=np.float32)
    norms = np.sqrt(ss)
    Xn8 = (X * (ESCALE / np.maximum(norms, np.float32(1e-12)))[:, None]).astype(NP_FP8)

    in_maps = _prep_core_inputs(Xn8, lab)

    if "nc" not in _CACHE:
        _CACHE["nc"] = _build_program()
    nc = _CACHE["nc"]

    trace = os.environ.get("BASS_KERNEL_TRACE", "0") == "1"
    res = run_bass_kernel_spmd(nc, in_maps, list(range(N_CORES)), trace=trace)
    LAST_RESULT = res

    # out[p, g]: group-major row r = g*128 + p
    maxneg = np.concatenate(
        [np.asarray(r["out"]).T.reshape(ROWS_PER_CORE) for r in res.results]
    )

    triplet = np.maximum(maxneg / SIM_SCALE + MARGIN, np.float32(0.0))
    has_neg = (np.asarray(labels) == 0).any(axis=1)
    w = (lab == 1) & np.repeat(has_neg, N)
    loss_sum = np.float32((triplet * w).sum(dtype=np.float64))
    count = int(w.sum())
    loss = np.float32(loss_sum / np.float32(max(count, 1)))
    return np.asarray(loss, dtype=np.float32)


# revision 6
# speedup vs baseline: 1.5698x; 1.0309x over previous
"""Trainium2 Bass kernel for CFContrastiveLoss.

Reference semantics (per sample of N=16 options, D=768 dims):
  - L2-normalize option embeddings
  - sim = pairwise cosine sims within the sample (16x16 gram)
  - max_neg[n] = max over negative-labeled columns of sim[n, :]
  - loss = mean over (positive rows of valid samples) of relu(max_neg + 0.3)

Device strategy (pure data parallel over batch, 8 cores):
  - 128 rows (= 8 samples x 16 options) per "group"; per core 16384 rows
    = 128 groups.  Groups are DMA'd in super-groups on the two HWDGE
    rings; the schedule is tapered (4,4,8,16,...,16,8,4,4) so the first
    PSUM bank's compute starts ~2us in and the post-last-byte compute
    tail is only 4 groups.  Embedding transfers are issued FIRST on
    each ring so nothing gates them.
  - Host pre-normalizes embeddings, scales by 16 (power of two; keeps
    elements inside e4m3's normal range) and casts to fp8 e4m3 in the
    matmul layout.  This is a memory-bound problem, so fp8 halves the
    HBM traffic vs fp16.  Per-sample gram matrices are computed on the
    TensorEngine as block-diagonal 128x128 grams (fp32 PSUM accumulate)
    using DoubleRow fp8 matmuls: each matmul consumes TWO 128-row
    k-subtiles at the double-pumped fp8 rate, so the 768-dim contraction
    is 3 matmuls instead of 6.  Sims come out scaled by 256; the host
    divides it back out.  e4m3 carries 3 mantissa bits; the per-sim
    error (~2e-3 absolute) averages out over the ~52k contributing rows
    and the max() bias stays small because top-sim gaps are larger than
    the noise.  Measured final loss error ~1.4e-4 (threshold 2e-2).
  - The label/validity masking is folded into the same PSUM accumulation
    as sentinel outer-product matmuls of +-2^14 (every factor is +-128
    or 0, all fp8-exact):
      row 0:  128*ones x negc       (negc[m] = -128 iff label[m] == 1)
      row 1:  128*ones x (-128*ones)  (mask everything ...)
      row 2+s: 128*u_s x (128*u_s)    (... except within-sample blocks)
    Sentinels are exact powers of two so they cancel exactly in fp32:
    in-block negative columns get an exactly-zero mask contribution and
    unmasked sims are bit-exact; masked entries sit at <= -2^14 + 256 so
    relu(max/256 + margin) = 0.  One 512-column mask matmul per PSUM
    bank covers 4 groups; it runs LAST in the bank's accumulation (the
    per-group grams open their column range with start=True) so the
    bank's compute can begin before the mask operands have landed.
    Mask operands are zero-padded to the full [128, 2, *] DoubleRow
    shape so every matmul runs the same tile config.
  - Per bank the device then does a single VectorE row-max from PSUM
    over a [128, 4, 128] view (reduces the innermost axis -> [128, 4]),
    amortizing DVE instruction overhead over 4 groups.  All row maxes
    accumulate in one [128, 128] SBUF tile, stored with a single DMA at
    the end.  relu/margin/weighting/final mean are O(rows) on host.
"""

import os

import ml_dtypes
import numpy as np

import concourse.bass as bass
import concourse.mybir as mybir
from concourse import bacc, tile
from concourse.bass_utils import run_bass_kernel_spmd

FP8 = mybir.dt.float8e4
NP_FP8 = ml_dtypes.float8_e4m3
F32 = mybir.dt.float32
DOUBLE_ROW = mybir.MatmulPerfMode.DoubleRow

B, N, D = 8192, 16, 768
N_CORES = 8
ROWS = B * N                      # 131072
ROWS_PER_CORE = ROWS // N_CORES   # 16384
GROUPS = ROWS_PER_CORE // 128     # 128 groups of 128 rows per core
KCH = D // 128                    # 6 contraction chunks
SG_MAX = 16                       # largest super-group (SBUF tile size)
GPB = 4                           # groups per PSUM bank (512 f32 / 128)
# Tapered schedule: fast ramp-in, long efficient middle, short tail.
SCHED = [4, 4, 8, 16, 16, 16, 16, 16, 16, 8, 4, 4]
assert sum(SCHED) == GROUPS
MASK_K = 2 + 128 // N             # 10 live mask matmul rows
SENT = np.float32(128.0)          # fp8-exact sentinel factor (128*128 = 2^14)
ESCALE = np.float32(16.0)         # fp8 embedding scale (power of two)
SIM_SCALE = ESCALE * ESCALE       # gram outputs are scaled by this
MARGIN = np.float32(0.3)

_CACHE: dict = {}

LAST_RESULT = None  # BassKernelResults of the most recent device run


def _build_program() -> bass.Bass:
    nc = bacc.Bacc(None)
    et = nc.declare_dram_parameter("et", [128, GROUPS * KCH, 128], FP8, isOutput=False)
    mrhs = nc.declare_dram_parameter("mrhs", [MASK_K, GROUPS * 128], FP8, isOutput=False)
    mlhs = nc.declare_dram_parameter("mlhs", [128, 128], FP8, isOutput=False)
    out = nc.declare_dram_parameter("out", [128, GROUPS], F32, isOutput=True)

    with tile.TileContext(nc) as tc:
        with (
            tc.tile_pool(name="emb", bufs=5) as emb_pool,
            tc.tile_pool(name="const", bufs=1) as const_pool,
            tc.tile_pool(name="psum", bufs=8, space="PSUM") as psum_pool,
        ):
            # Mask lhsT as a [128, 2, 128] DoubleRow weight tensor;
            # k-subtile 1 is all zeros (contributes nothing).
            mlhs_t = const_pool.tile([128, 2, 128], FP8)
            nc.vector.memset(mlhs_t[:, :, :], 0.0)
            nc.scalar.dma_start(mlhs_t[:, 0, :], mlhs[:])
            # Two ping-pong mask-rhs tiles; rows (MASK_K..127) of subtile
            # 0 and all of subtile 1 stay zero -- memset once (on two
            # different engines, concurrently; they are off the critical
            # path of the embedding stream) so the padded lanes
            # contribute nothing (and no NaN*0).
            mr_tiles = []
            for i, eng in ((0, nc.vector), (1, nc.gpsimd)):
                mr_t = const_pool.tile([128, 2, SG_MAX * 128], FP8, name=f"mr{i}")
                eng.memset(mr_t[:, :, :], 0.0)
                mr_tiles.append(mr_t)
            # All 128 row-maxes accumulate here; one DMA at the end.
            wide = const_pool.tile([128, GROUPS], F32)

            g0 = 0
            for sg, ng in enumerate(SCHED):
                hi = emb_pool.tile([128, SG_MAX * KCH, 128], FP8, tag="hi")
                mr_t = mr_tiles[sg % 2]
                nk = ng * KCH
                c0 = g0 * KCH
                # Embedding transfers go FIRST on their ring so the mask
                # loads never gate them; split across both HWDGE rings.
                if ng >= 8:
                    hc = nk // 2
                    nc.sync.dma_start(hi[:, :hc, :], et[:, c0:c0 + hc, :])
                    nc.scalar.dma_start(hi[:, hc:nk, :], et[:, c0 + hc:c0 + nk, :])
                    nc.scalar.dma_start(
                        mr_t[:MASK_K, 0, :ng * 128],
                        mrhs[:, g0 * 128:(g0 + ng) * 128])
                else:
                    ring = nc.sync if sg % 2 == 0 else nc.scalar
                    oring = nc.scalar if sg % 2 == 0 else nc.sync
                    ring.dma_start(hi[:, :nk, :], et[:, c0:c0 + nk, :])
                    oring.dma_start(
                        mr_t[:MASK_K, 0, :ng * 128],
                        mrhs[:, g0 * 128:(g0 + ng) * 128])
                for b in range(ng // GPB):
                    ps = psum_pool.tile([128, GPB, 128], F32)  # one full PSUM bank
                    for g in range(GPB):
                        gi = b * GPB + g
                        G = ps[:, g, :]
                        for k in range(KCH // 2):
                            kc = gi * KCH + 2 * k
                            hk = hi[:, kc:kc + 2, :]
                            # start=True marks the WHOLE bank pending-zero
                            # (per-element has_written bits), so only the
                            # bank's first matmul sets it; later groups'
                            # first writes land on pending-zero bytes and
                            # overwrite, everything else accumulates.
                            nc.tensor.matmul(
                                G, hk, hk,
                                start=(g == 0 and k == 0), stop=False,
                                perf_mode=DOUBLE_ROW,
                            )
                    # Mask sentinels for the whole bank, accumulated LAST
                    # so bank compute can start before mask operands land.
                    mc = b * GPB * 128
                    nc.tensor.matmul(
                        ps[:, :, :], mlhs_t[:, :, :], mr_t[:, :, mc:mc + GPB * 128],
                        start=False, stop=True, perf_mode=DOUBLE_ROW,
                    )
                    # One DVE reduce for the whole bank: [128, 4, 128]
                    # reduced over the innermost axis -> [128, 4].
                    nc.vector.reduce_max(
                        wide[:, g0 + b * GPB: g0 + (b + 1) * GPB],
                        ps[:, :, :], axis=mybir.AxisListType.X)
                g0 += ng
            nc.sync.dma_start(out[:, :], wide[:, :])
    nc.finalize()
    return nc


def _prep_core_inputs(Xn8: np.ndarray, lab: np.ndarray):
    """Per-core input maps from fp8-normalized embeddings + flat labels."""
    negc = np.where(lab == 1, -SENT, np.float32(0.0)).astype(NP_FP8)

    m_idx = np.arange(128)
    # mask lhsT: [128, 128], rows = SENT * [ones; ones; u_0..u_7; zeros...]
    mlhs = np.zeros((128, 128), dtype=NP_FP8)
    mlhs[0, :] = SENT
    mlhs[1, :] = SENT
    for s in range(128 // N):
        mlhs[2 + s, :] = (SENT * (m_idx // N == s)).astype(NP_FP8)

    # static part of mask rhs rows 1..9 (per 128-column group)
    mrhs_static = np.zeros((MASK_K, 128), dtype=NP_FP8)
    mrhs_static[1, :] = -SENT
    for s in range(128 // N):
        mrhs_static[2 + s, :] = np.where(m_idx // N == s, SENT, np.float32(0.0)).astype(NP_FP8)

    def to_layout(a_core: np.ndarray) -> np.ndarray:
        # [rows=16384, 768] -> [g, n, k, p] -> [p, g, k, n]
        return np.ascontiguousarray(
            a_core.reshape(GROUPS, 128, KCH, 128).transpose(3, 0, 2, 1)
        ).reshape(128, GROUPS * KCH, 128)

    in_maps = []
    for c in range(N_CORES):
        r0 = c * ROWS_PER_CORE
        r1 = r0 + ROWS_PER_CORE
        mr = np.empty((MASK_K, ROWS_PER_CORE), dtype=NP_FP8)
        mr[0, :] = negc[r0:r1]
        mr[1:, :] = np.tile(mrhs_static[1:, :], (1, GROUPS))
        in_maps.append({
            "et": to_layout(Xn8[r0:r1]),
            "mrhs": mr,
            "mlhs": mlhs,
        })
    return in_maps


def kernel(embeddings: np.ndarray, labels: np.ndarray) -> np.ndarray:
    global LAST_RESULT
    assert embeddings.shape == (B, N, D)
    assert labels.shape == (B, N)

    X = np.asarray(embeddings, dtype=np.float32).reshape(ROWS, D)
    lab = np.asarray(labels).reshape(ROWS)

    ss = np.square(X).sum(axis=1, dtype=np.float32)
    norms = np.sqrt(ss)
    Xn8 = (X * (ESCALE / np.maximum(norms, np.float32(1e-12)))[:, None]).astype(NP_FP8)

    in_maps = _prep_core_inputs(Xn8, lab)

    if "nc" not in _CACHE:
        _CACHE["nc"] = _build_program()
    nc = _CACHE["nc"]

    trace = os.environ.get("BASS_KERNEL_TRACE", "0") == "1"
    res = run_bass_kernel_spmd(nc, in_maps, list(range(N_CORES)), trace=trace)
    LAST_RESULT = res

    # out[p, g]: group-major row r = g*128 + p
    maxneg = np.concatenate(
        [np.asarray(r["out"]).T.reshape(ROWS_PER_CORE) for r in res.results]
    )

    triplet = np.maximum(maxneg / SIM_SCALE + MARGIN, np.float32(0.0))
    has_neg = (np.asarray(labels) == 0).any(axis=1)
    w = (lab == 1) & np.repeat(has_neg, N)
    loss_sum = np.float32((triplet * w).sum(dtype=np.float64))
    count = int(w.sum())
    loss = np.float32(loss_sum / np.float32(max(count, 1)))
    return np.asarray(loss, dtype=np.float32)
